# revision 4
# baseline (speedup 1.0000x reference)
"""Chamfer distance kernel for 8 Trainium2 NeuronCores.

Problem: x, y: [4, 8192, 3] f32 point clouds.
  D[b,i,j] = ||x[b,i] - y[b,j]||^2
  out = mean_{b,i} min_j sqrt(D) + mean_{b,j} min_i sqrt(D)

v4 strategy (vs baseline: fp32 K=5 matmul, fp16 min ops):
  - fp16 K=13 error-compensated matmul computing NEGATED distances -D:
    rows [-xx_hi, -xx_lo, -1, -1, 2x_hi, 2x_lo, 2x_hi] x
         [1, 1, yy_hi, yy_lo, y_hi, y_hi, y_lo].
    Measured on HW: fp16-split matches fp32 K=5 precision (rms error
    ~7e-7 in D, critical because true nearest-neighbor D minima are
    ~1e-4 for these clouds) while the PE runs at 1 cycle/row, 4x faster
    than fp32. (bf16-split: 1.9e-5 rms - too noisy; fp32r: garbage.)
  - Negation matters: DVE tensor_tensor fp16 MAX runs in 2x mode
    (measured 2279ns for [128,4096]) while MIN runs 1x (4092ns). All
    reductions become max; host negates at the end.
  - 4-way PE row-quadrant packing (tile_position 0/32/64/96), K=13<=32.
  - ACT drains every PSUM span to fp16. DVE runs per-chunk row-max
    trees on CONTIGUOUS slices (strided APs lose the 2x mode) down to
    512 wide; the [128,512] row partials are DMA'd to DRAM and folded
    on host, skipping the slow 1x on-device tensor_reduce.
  - Col direction: DVE fp16 max accumulation for most chunks; the Pool
    engine takes POOL_COL chunks via partition_all_reduce (measured
    ~15us per [128,4096]) with per-chunk results DMA'd to DRAM.
  - Sharding: 8 cores = 4 batches x 2 j-halves; each core owns an
    [8192, 4096] block of the distance matrix.
"""

import sys

if "/opt/trn_rl_repo" not in sys.path:
    sys.path.insert(0, "/opt/trn_rl_repo")

import numpy as np


def _install_ntff_hook_shim():
    """The agent image's antenv lacks axon_hooks; bass_utils imports it when
    BASS_TRACE is set. Register a stand-in backed by the ctypes NTFF hook."""
    import types

    if "antenv.axon_hooks" in sys.modules:
        return
    try:
        import antenv
        from trn_agent_boot.trn_boot import _ntff_profile_via_ctypes
    except ImportError:
        return
    mod = types.ModuleType("antenv.axon_hooks")
    _hook = [None]

    def set_axon_ntff_profile_hook(h):
        _hook[0] = h

    def get_axon_ntff_profile_hook():
        if _hook[0] is None:
            try:
                _hook[0] = _ntff_profile_via_ctypes("/opt/axon/libaxon_pjrt.so")
            except Exception:
                return None
        return _hook[0]

    mod.set_axon_ntff_profile_hook = set_axon_ntff_profile_hook
    mod.get_axon_ntff_profile_hook = get_axon_ntff_profile_hook
    sys.modules["antenv.axon_hooks"] = mod
    antenv.axon_hooks = mod


_install_ntff_hook_shim()

import concourse.bacc as bacc
import concourse.bass as bass
import concourse.bass_isa as bass_isa
import concourse.mybir as mybir
import concourse.tile as tile
from concourse.bass_utils import run_bass_kernel_spmd

BS = 4
N = 8192
K = 13                 # fp16-split contraction rows
NCHUNKS = 64           # i-chunks of 128 rows
NPAIRS = NCHUNKS // 2
NJT = 8                # j-tiles of 512 cols per core (half of 8192)
JH = NJT * 512         # 4096 columns per core

N_CORES = 8

F32 = mybir.dt.float32
F16 = mybir.dt.float16
MAX_OP = mybir.AluOpType.max
COPY_FN = mybir.ActivationFunctionType.Copy

# chunks whose col direction runs on the Pool engine (partition_all_reduce),
# by chunk index mod 4 == 1 -> 16 of 64 chunks. The rest accumulate on DVE.
def _is_pool_chunk(c):
    return (c % 4) == 1


LAST_RESULTS = None
_compiled_nc = None


def _build_program():
    nc = bacc.Bacc()

    xa = nc.declare_dram_parameter("xa", [K, N], F16, isOutput=False)
    ya = nc.declare_dram_parameter("ya", [K, JH], F16, isOutput=False)
    # row partials: [128, 512] of -D per chunk, host max-reduces + negates
    rowpart_out = nc.declare_dram_parameter("rowpart", [128, NCHUNKS, 512], F16, isOutput=True)
    colmax_out = nc.declare_dram_parameter("colmax", [128, NJT, 512], F16, isOutput=True)
    colsmall_out = nc.declare_dram_parameter("colsmall", [NCHUNKS, JH], F16, isOutput=True)

    with tile.TileContext(nc) as tc:
        with (
            tc.tile_pool(name="const", bufs=1) as const_pool,
            tc.tile_pool(name="acc", bufs=1) as acc_pool,
            tc.tile_pool(name="d16", bufs=4) as d16_pool,
            tc.tile_pool(name="scr", bufs=3) as scr_pool,
            tc.tile_pool(name="ar", bufs=2) as ar_pool,
            tc.tile_pool(name="psum", bufs=2, space="PSUM") as psum_pool,
        ):
            # xa/ya replicated at partition offsets 0/32/64/96 so four K=13
            # matmuls run in distinct PE row-quadrants.
            xa_sb = const_pool.tile([96 + K, N], F16, tag="xa")
            ya_sb = const_pool.tile([96 + K, JH], F16, tag="ya")
            # prefetch order: first chunks'/spans' slices land first.
            for m in range(4):
                nc.sync.dma_start(xa_sb[32 * m:32 * m + K, 0:512], xa[:, 0:512])
            for m in range(4):
                nc.sync.dma_start(ya_sb[32 * m:32 * m + K, 0:2048], ya[:, 0:2048])
            for m in range(4):
                nc.sync.dma_start(ya_sb[32 * m:32 * m + K, 2048:], ya[:, 2048:])
            for m in range(4):
                nc.sync.dma_start(xa_sb[32 * m:32 * m + K, 512:], xa[:, 512:])

            colacc = acc_pool.tile([128, NJT, 512], F16, tag="colacc")

            first_dve_col = [True]

            for p in range(NPAIRS):
                # d16: [cc, span, 2048] fp16 of -D for this chunk pair
                d16 = d16_pool.tile([128, 2, 2, 2048], F16)
                scr = scr_pool.tile([128, 2, 2048], F16)
                for cc in range(2):
                    c = 2 * p + cc
                    for s in range(2):
                        ps = psum_pool.tile([128, 4, 512], F32)
                        for m in range(4):
                            t = s * 4 + m
                            nc.tensor.matmul(
                                ps[:, m, :],
                                xa_sb[32 * m:32 * m + K, c * 128:(c + 1) * 128],
                                ya_sb[32 * m:32 * m + K, t * 512:(t + 1) * 512],
                                start=True, stop=True,
                                tile_position=(32 * m, 0),
                            )
                        nc.scalar.activation(
                            d16[:, cc, s].rearrange("p f -> p f"), ps[:], COPY_FN
                        )

                    dchunk = d16[:, cc].rearrange("p s f -> p (s f)")  # [128,4096]
                    # column direction for chunk c (-D maxes, (p,j)-resolved)
                    if _is_pool_chunk(c):
                        arr = ar_pool.tile([128, JH], F16)
                        nc.gpsimd.partition_all_reduce(
                            arr[:], dchunk, 128, bass_isa.ReduceOp.max
                        )
                        nc.sync.dma_start(colsmall_out[c:c + 1, :], arr[0:1, :])
                    else:
                        ca = colacc[:].rearrange("p jt f -> p (jt f)")
                        if first_dve_col[0]:
                            nc.vector.tensor_copy(ca, dchunk)
                            first_dve_col[0] = False
                        else:
                            nc.vector.tensor_tensor(ca, ca, dchunk, MAX_OP)

                    # row direction: contiguous fp16 2x max-tree to 512 wide,
                    # then DMA the partial out; host finishes the reduce.
                    sc = scr[:, cc]
                    nc.vector.tensor_tensor(sc, d16[:, cc, 0], d16[:, cc, 1], MAX_OP)
                    nc.vector.tensor_tensor(
                        sc[:, 0:1024], sc[:, 0:1024], sc[:, 1024:2048], MAX_OP
                    )
                    nc.vector.tensor_tensor(
                        sc[:, 0:512], sc[:, 0:512], sc[:, 512:1024], MAX_OP
                    )
                    nc.sync.dma_start(rowpart_out[:, c, :], sc[:, 0:512])

            nc.sync.dma_start(colmax_out[:], colacc[:])

    nc.compile()
    return nc


def _augment(x, y):
    """fp16-split augmentation for NEGATED distances.

    xaugT[b]: [13, N] rows (-xx_hi, -xx_lo, -1, -1, 2x_hi, 2x_lo, 2x_hi)
    yaugT[b]: [13, N] rows (1, 1, yy_hi, yy_lo, y_hi, y_hi, y_lo)
    Sum over rows = -(xx + yy - 2(x_hi.y_hi + x_lo.y_hi + x_hi.y_lo)) ~= -D.
    """
    f16 = np.float16
    x = np.asarray(x, dtype=np.float32)
    y = np.asarray(y, dtype=np.float32)

    def split(v):
        hi = v.astype(f16).astype(np.float32)
        lo = (v - hi).astype(f16).astype(np.float32)
        return hi, lo

    xx = (x.astype(np.float64) ** 2).sum(-1).astype(np.float32)  # [b, n]
    yy = (y.astype(np.float64) ** 2).sum(-1).astype(np.float32)
    xxh, xxl = split(xx)
    yyh, yyl = split(yy)
    xh, xl = split(x)   # [b, n, 3]
    yh, yl = split(y)
    ones = np.ones_like(xx)

    xrows = [-xxh, -xxl, -ones, -ones]
    yrows = [ones, ones, yyh, yyl]
    for d in range(3):
        xrows.append(2.0 * xh[..., d])
        yrows.append(yh[..., d])
    for d in range(3):
        xrows.append(2.0 * xl[..., d])
        yrows.append(yh[..., d])
    for d in range(3):
        xrows.append(2.0 * xh[..., d])
        yrows.append(yl[..., d])

    xaug = np.stack(xrows, axis=1).astype(f16)  # [b, 13, n]
    yaug = np.stack(yrows, axis=1).astype(f16)
    return xaug, yaug


def kernel(x, y):
    global LAST_RESULTS, _compiled_nc

    x = np.asarray(x, dtype=np.float32)
    y = np.asarray(y, dtype=np.float32)
    bs, n, d = x.shape
    assert (bs, n, d) == (BS, N, 3), (bs, n, d)

    xaug, yaug = _augment(x, y)  # [4, 13, 8192] fp16 each

    in_maps = []
    for core in range(N_CORES):
        b, h = divmod(core, 2)
        in_maps.append({
            "xa": np.ascontiguousarray(xaug[b]),
            "ya": np.ascontiguousarray(yaug[b][:, h * JH:(h + 1) * JH]),
        })

    if _compiled_nc is None:
        _compiled_nc = _build_program()

    res = None
    last_err = None
    for attempt in range(3):
        try:
            res = run_bass_kernel_spmd(_compiled_nc, in_maps, list(range(N_CORES)))
            break
        except Exception as e:  # transient axon/NRT hiccups: rebuild + retry
            last_err = e
            _compiled_nc = _build_program()
    if res is None:
        raise last_err
    LAST_RESULTS = res

    pool_rows = np.array([c for c in range(NCHUNKS) if _is_pool_chunk(c)])

    vals1_sq = np.empty((BS, N), dtype=np.float32)
    vals2_sq = np.empty((BS, N), dtype=np.float32)
    for b in range(BS):
        # row: [128, 64, 512] fp16 of -D partials per j-half; fold both
        rp0 = res.results[2 * b]["rowpart"].astype(np.float32).max(axis=2)
        rp1 = res.results[2 * b + 1]["rowpart"].astype(np.float32).max(axis=2)
        rm = np.maximum(rp0, rp1)               # [128, 64] max of -D
        # i = c*128 + p  ->  [64, 128] row-major flatten; negate -> min(D)
        vals1_sq[b] = -rm.T.reshape(-1)
        for h in range(2):
            r = res.results[2 * b + h]
            ca = r["colmax"].astype(np.float32)       # [128, 8, 512] -D maxes
            cs = r["colsmall"].astype(np.float32)     # [64, 4096] pool chunks
            cmax = np.maximum(
                ca.reshape(128, -1).max(axis=0),
                cs[pool_rows].max(axis=0),
            )
            vals2_sq[b, h * JH:(h + 1) * JH] = -cmax

    vals1 = np.sqrt(np.maximum(vals1_sq, 0.0))
    vals2 = np.sqrt(np.maximum(vals2_sq, 0.0))
    out = vals1.mean(axis=1).mean() + vals2.mean(axis=1).mean()
    return np.float32(out)


# revision 5
# speedup vs baseline: 1.4477x; 1.4477x over previous
"""Chamfer distance kernel for 8 Trainium2 NeuronCores.

Problem: x, y: [4, 8192, 3] f32 point clouds.
  D[b,i,j] = ||x[b,i] - y[b,j]||^2
  out = mean_{b,i} min_j sqrt(D) + mean_{b,j} min_i sqrt(D)

v5 strategy (vs baseline: fp32 K=5 matmul, fp16 min ops):
  - fp16 K=13 error-compensated matmul computing NEGATED distances -D:
    rows [-xx_hi, -xx_lo, -1, -1, 2x_hi, 2x_lo, 2x_hi] x
         [1, 1, yy_hi, yy_lo, y_hi, y_hi, y_lo].
    Measured on HW: fp16-split matches fp32 K=5 precision (rms error
    ~7e-7 in D, critical because true nearest-neighbor D minima are
    ~1e-4 for these clouds) while the PE runs at 1 cycle/row, 4x faster
    than fp32. (bf16-split: 1.9e-5 rms - too noisy; fp32r: garbage.)
  - Negation matters: DVE tensor_tensor fp16 MAX runs in 2x mode
    (measured 2279ns for [128,4096]) while MIN runs 1x (4092ns). All
    reductions become max; host negates at the end.
  - 4-way PE row-quadrant packing (tile_position 0/32/64/96), K=13<=32.
  - ACT drains every PSUM span to fp16 (253us - the bottleneck engine).
    DVE does exactly 2 contiguous fp16 2x ops per chunk: one row-pair
    max (spans folded, [128,2048]) and one col accumulation
    ([128,4096]). Row partials go to DRAM via the idle DMA engines and
    the host finishes the 2048->1 max-reduce (device reduces are 1x and
    cost more than they save).
  - gpsimd partition_all_reduce was tried for the col direction and
    REVERTED: its SBUF traffic starves concurrent DVE ops on the same
    tile (tree ops ballooned 1.2us -> 8us).
  - Sharding: 8 cores = 4 batches x 2 j-halves; each core owns an
    [8192, 4096] block of the distance matrix.
"""

import sys

if "/opt/trn_rl_repo" not in sys.path:
    sys.path.insert(0, "/opt/trn_rl_repo")

import numpy as np


def _install_ntff_hook_shim():
    """The agent image's antenv lacks axon_hooks; bass_utils imports it when
    BASS_TRACE is set. Register a stand-in backed by the ctypes NTFF hook."""
    import types

    if "antenv.axon_hooks" in sys.modules:
        return
    try:
        import antenv
        from trn_agent_boot.trn_boot import _ntff_profile_via_ctypes
    except ImportError:
        return
    mod = types.ModuleType("antenv.axon_hooks")
    _hook = [None]

    def set_axon_ntff_profile_hook(h):
        _hook[0] = h

    def get_axon_ntff_profile_hook():
        if _hook[0] is None:
            try:
                _hook[0] = _ntff_profile_via_ctypes("/opt/axon/libaxon_pjrt.so")
            except Exception:
                return None
        return _hook[0]

    mod.set_axon_ntff_profile_hook = set_axon_ntff_profile_hook
    mod.get_axon_ntff_profile_hook = get_axon_ntff_profile_hook
    sys.modules["antenv.axon_hooks"] = mod
    antenv.axon_hooks = mod


_install_ntff_hook_shim()

import concourse.bacc as bacc
import concourse.bass as bass
import concourse.mybir as mybir
import concourse.tile as tile
from concourse.bass_utils import run_bass_kernel_spmd

BS = 4
N = 8192
K = 13                 # fp16-split contraction rows
NCHUNKS = 64           # i-chunks of 128 rows
NPAIRS = NCHUNKS // 2
NJT = 8                # j-tiles of 512 cols per core (half of 8192)
JH = NJT * 512         # 4096 columns per core

N_CORES = 8

F32 = mybir.dt.float32
F16 = mybir.dt.float16
MAX_OP = mybir.AluOpType.max
COPY_FN = mybir.ActivationFunctionType.Copy

LAST_RESULTS = None
_compiled_nc = None


def _build_program():
    nc = bacc.Bacc()

    xa = nc.declare_dram_parameter("xa", [K, N], F16, isOutput=False)
    ya = nc.declare_dram_parameter("ya", [K, JH], F16, isOutput=False)
    # row partials: [128, 2048] of -D per chunk; host max-reduces + negates
    rowpart_out = nc.declare_dram_parameter("rowpart", [128, NCHUNKS, 2048], F16, isOutput=True)
    colmax_out = nc.declare_dram_parameter("colmax", [128, NJT, 512], F16, isOutput=True)

    with tile.TileContext(nc) as tc:
        with (
            tc.tile_pool(name="const", bufs=1) as const_pool,
            tc.tile_pool(name="acc", bufs=1) as acc_pool,
            tc.tile_pool(name="d16", bufs=4) as d16_pool,
            tc.tile_pool(name="scr", bufs=3) as scr_pool,
            tc.tile_pool(name="psum", bufs=2, space="PSUM") as psum_pool,
        ):
            # xa/ya replicated at partition offsets 0/32/64/96 so four K=13
            # matmuls run in distinct PE row-quadrants.
            xa_sb = const_pool.tile([96 + K, N], F16, tag="xa")
            ya_sb = const_pool.tile([96 + K, JH], F16, tag="ya")
            # prefetch order: first chunks'/spans' slices land first.
            for m in range(4):
                nc.sync.dma_start(xa_sb[32 * m:32 * m + K, 0:512], xa[:, 0:512])
            for m in range(4):
                nc.sync.dma_start(ya_sb[32 * m:32 * m + K, 0:2048], ya[:, 0:2048])
            for m in range(4):
                nc.sync.dma_start(ya_sb[32 * m:32 * m + K, 2048:], ya[:, 2048:])
            for m in range(4):
                nc.sync.dma_start(xa_sb[32 * m:32 * m + K, 512:], xa[:, 512:])

            colacc = acc_pool.tile([128, NJT, 512], F16, tag="colacc")

            for p in range(NPAIRS):
                # d16: [cc, span, 2048] fp16 of -D for this chunk pair
                d16 = d16_pool.tile([128, 2, 2, 2048], F16)
                scr = scr_pool.tile([128, 2, 2048], F16)
                for cc in range(2):
                    c = 2 * p + cc
                    for s in range(2):
                        ps = psum_pool.tile([128, 4, 512], F32)
                        for m in range(4):
                            t = s * 4 + m
                            nc.tensor.matmul(
                                ps[:, m, :],
                                xa_sb[32 * m:32 * m + K, c * 128:(c + 1) * 128],
                                ya_sb[32 * m:32 * m + K, t * 512:(t + 1) * 512],
                                start=True, stop=True,
                                tile_position=(32 * m, 0),
                            )
                        nc.scalar.activation(
                            d16[:, cc, s].rearrange("p f -> p f"), ps[:], COPY_FN
                        )

                    # row direction: one contiguous fp16 2x span-pair max;
                    # host finishes the 2048-wide reduce from DRAM.
                    sc = scr[:, cc]
                    nc.vector.tensor_tensor(sc, d16[:, cc, 0], d16[:, cc, 1], MAX_OP)
                    nc.sync.dma_start(rowpart_out[:, c, :], sc)

                    # column direction: one contiguous fp16 2x accumulation
                    dchunk = d16[:, cc].rearrange("p s f -> p (s f)")  # [128,4096]
                    ca = colacc[:].rearrange("p jt f -> p (jt f)")
                    if p == 0 and cc == 0:
                        nc.vector.tensor_copy(ca, dchunk)
                    else:
                        nc.vector.tensor_tensor(ca, ca, dchunk, MAX_OP)

            nc.sync.dma_start(colmax_out[:], colacc[:])

    nc.compile()
    return nc


def _augment(x, y):
    """fp16-split augmentation for NEGATED distances.

    xaugT[b]: [13, N] rows (-xx_hi, -xx_lo, -1, -1, 2x_hi, 2x_lo, 2x_hi)
    yaugT[b]: [13, N] rows (1, 1, yy_hi, yy_lo, y_hi, y_hi, y_lo)
    Sum over rows = -(xx + yy - 2(x_hi.y_hi + x_lo.y_hi + x_hi.y_lo)) ~= -D.
    """
    f16 = np.float16
    x = np.asarray(x, dtype=np.float32)
    y = np.asarray(y, dtype=np.float32)

    def split(v):
        hi = v.astype(f16).astype(np.float32)
        lo = (v - hi).astype(f16).astype(np.float32)
        return hi, lo

    xx = (x.astype(np.float64) ** 2).sum(-1).astype(np.float32)  # [b, n]
    yy = (y.astype(np.float64) ** 2).sum(-1).astype(np.float32)
    xxh, xxl = split(xx)
    yyh, yyl = split(yy)
    xh, xl = split(x)   # [b, n, 3]
    yh, yl = split(y)
    ones = np.ones_like(xx)

    xrows = [-xxh, -xxl, -ones, -ones]
    yrows = [ones, ones, yyh, yyl]
    for d in range(3):
        xrows.append(2.0 * xh[..., d])
        yrows.append(yh[..., d])
    for d in range(3):
        xrows.append(2.0 * xl[..., d])
        yrows.append(yh[..., d])
    for d in range(3):
        xrows.append(2.0 * xh[..., d])
        yrows.append(yl[..., d])

    xaug = np.stack(xrows, axis=1).astype(f16)  # [b, 13, n]
    yaug = np.stack(yrows, axis=1).astype(f16)
    return xaug, yaug


def kernel(x, y):
    global LAST_RESULTS, _compiled_nc

    x = np.asarray(x, dtype=np.float32)
    y = np.asarray(y, dtype=np.float32)
    bs, n, d = x.shape
    assert (bs, n, d) == (BS, N, 3), (bs, n, d)

    xaug, yaug = _augment(x, y)  # [4, 13, 8192] fp16 each

    in_maps = []
    for core in range(N_CORES):
        b, h = divmod(core, 2)
        in_maps.append({
            "xa": np.ascontiguousarray(xaug[b]),
            "ya": np.ascontiguousarray(yaug[b][:, h * JH:(h + 1) * JH]),
        })

    if _compiled_nc is None:
        _compiled_nc = _build_program()

    res = None
    last_err = None
    for attempt in range(3):
        try:
            res = run_bass_kernel_spmd(_compiled_nc, in_maps, list(range(N_CORES)))
            break
        except Exception as e:  # transient axon/NRT hiccups: rebuild + retry
            last_err = e
            _compiled_nc = _build_program()
    if res is None:
        raise last_err
    LAST_RESULTS = res

    vals1_sq = np.empty((BS, N), dtype=np.float32)
    vals2_sq = np.empty((BS, N), dtype=np.float32)
    for b in range(BS):
        # row partials: [128, 64, 2048] fp16 of -D per j-half; reduce + fold
        rp0 = res.results[2 * b]["rowpart"].astype(np.float32).max(axis=2)
        rp1 = res.results[2 * b + 1]["rowpart"].astype(np.float32).max(axis=2)
        rm = np.maximum(rp0, rp1)               # [128, 64] max of -D
        # i = c*128 + p  ->  [64, 128] row-major flatten; negate -> min(D)
        vals1_sq[b] = -rm.T.reshape(-1)
        for h in range(2):
            ca = res.results[2 * b + h]["colmax"].astype(np.float32)
            vals2_sq[b, h * JH:(h + 1) * JH] = -ca.reshape(128, -1).max(axis=0)

    vals1 = np.sqrt(np.maximum(vals1_sq, 0.0))
    vals2 = np.sqrt(np.maximum(vals2_sq, 0.0))
    out = vals1.mean(axis=1).mean() + vals2.mean(axis=1).mean()
    return np.float32(out)
